# revision 24
# baseline (speedup 1.0000x reference)
"""Trainium2 Bass kernel for nn_DeepSSM: LSTM over [B=256, T=2048, obs=32]
-> [B, T, 64], data-parallel over 8 NeuronCores (32 batch each).

Design (one core, all 32 batch in one vectorized chain)
-------------------------------------------------------
The wall-clock of an LSTM on TRN2 is dominated by the per-timestep
dependency chain h_{t-1} -> gates_t -> h_t; per-instruction fixed costs
(~0.4-1us) dwarf the actual math, so the kernel minimizes instructions on
that chain:

* ONE recurrent matmul per step via a block-diagonal stationary matrix.
  The 256 gate outputs are split into chunk A = [i; o] and chunk B =
  [f; g] (128 psum partitions each). lhsT [128, 128] holds Wh columns
  for chunk B in rows 0:64 and chunk A in rows 64:128; the moving slot
  [128, 64] bf16 carries h at rows 64:128 of the first 32 cols (-> chunk
  A gates) and at rows 0:64 of the next 32 (-> chunk B), zeros elsewhere
  (memset once). Psum writes advance sequentially through a bank
  (8 steps x 64 cols = one 2KB bank), which profiled far faster than
  scattered psum targets.

* x-projections + bias are batched: one matmul per bank fills all 512
  cols from a staged rhs [66, 512] ([x;1] duplicated into the two
  block-diagonal quadrants by the host/DMA, zeros elsewhere).

* Real sigmoid/tanh (both live in the same ACT table -> no table
  reloads). Per step:
    ACT  sigmoid over all gates -> T [128, 2B] (sig_i | sig_f on p<64,
         sig_o | junk on p>=64)
    ACT  tanh(g-hat) -> X even slots          (X: t_g@even, c@odd)
    DVE  M = (sig_i*t_g @even, sig_f*c @odd)  (one contiguous mult)
    DVE  pairwise scan d0=[0,1]: c' = sig_i*t_g + sig_f*c -> X' odd slots
    ACT  w = tanh(c') -> partitions 64:128
    DVE  h = sig_o * w -> next h-slot (A quadrant), then DVE copy to the
         B quadrant.
  The scan writes X' directly (even slots hold dead partial sums that
  the next step's tanh(g) overwrites), keeping the cell state layout
  self-consistent with zero shuffle instructions.

* h-slots double as output staging: one DMA per 256-step chunk reads the
  A-quadrant columns back to DRAM (bf16), so the serial chain carries no
  extra stores. x is staged 2 chunks ahead in two strided DMAs per
  chunk. Total DMA count is ~35 per core.

The host pre-transposes x (adding the ones-row for the bias, duplicated
for the two quadrants) and post-transposes the bf16 output.
"""

import os
import numpy as np
import ml_dtypes

BF16 = ml_dtypes.bfloat16

OBS = 32
HID = 64
T_FULL = 2048
B_FULL = 256
N_CORES = 8
BPC = B_FULL // N_CORES   # 32 batch per core
KA = OBS + 1              # x rows incl ones-row
WIN2 = 8                  # steps per psum bank (8 * 64 cols = 512 f32)

_NC_CACHE = {}


def build_nc2(t_steps=T_FULL):
    import concourse.bass as bass
    import concourse.tile as tile
    import concourse.mybir as mybir

    S = min(256, t_steps)     # steps per staging chunk
    f32 = mybir.dt.float32
    bf16 = mybir.dt.bfloat16
    TANH = mybir.ActivationFunctionType.Tanh
    SIG = mybir.ActivationFunctionType.Sigmoid
    MULT = mybir.AluOpType.mult
    ADD = mybir.AluOpType.add

    B = BPC
    n_chunk = t_steps // S
    n_bank = S // WIN2
    nc = bass.Bass("TRN2", debug=False, num_devices=N_CORES,
                   enable_partition_id=False)

    xcat = nc.dram_tensor("xcat", [2 * KA, t_steps, B], bf16,
                          kind="ExternalInput")
    wall = nc.dram_tensor("wall", [128, 256], bf16, kind="ExternalInput")
    hout = nc.dram_tensor("hout", [HID, t_steps, B], bf16,
                          kind="ExternalOutput")

    with tile.TileContext(nc) as tc:
        from contextlib import ExitStack
        ctx = ExitStack()
        with ctx:
            wpool = ctx.enter_context(tc.tile_pool(name="w", bufs=1))
            tpool = ctx.enter_context(tc.tile_pool(name="T", bufs=4))
            xpool = ctx.enter_context(tc.tile_pool(name="X", bufs=4))
            mpool = ctx.enter_context(tc.tile_pool(name="M", bufs=4))
            wtpool = ctx.enter_context(tc.tile_pool(name="wt", bufs=4))
            rhsp = ctx.enter_context(tc.tile_pool(name="rhs", bufs=1))
            xsp = ctx.enter_context(tc.tile_pool(name="xs", bufs=1))
            bankp = ctx.enter_context(
                tc.tile_pool(name="bank", bufs=2, space="PSUM"))

            w_all = wpool.tile([128, 256], bf16)
            nc.sync.dma_start(w_all[:, :], wall[:, :])
            wh_ap = w_all[:, 0:128]                  # block-diag [WhB; WhA]
            wx_ap = w_all[0:2 * KA, 128:256]         # [66, 128] [WxB; WxA]

            # scan multiplier: 0 at even (reset to S_i), 1 at odd (add S_f)
            scanc = wpool.tile([HID, 2 * B], f32)
            nc.vector.memset(scanc[:, :].rearrange(
                "p (b c) -> p c b", c=2)[:, 0, :], 0.0)
            nc.vector.memset(scanc[:, :].rearrange(
                "p (b c) -> p c b", c=2)[:, 1, :], 1.0)

            # persistent double-buffered h-slot / x staging regions
            rhs_bufs = [rhsp.tile([128, S * 2 * B], bf16, name=f"rhsb{i}")
                        for i in range(2)]
            xs_bufs = [xsp.tile([2 * KA, S * 2 * B], bf16, name=f"xsb{i}")
                       for i in range(2)]
            for i in range(2):
                nc.vector.memset(rhs_bufs[i][:, :], 0.0)
                nc.gpsimd.memset(xs_bufs[i][:, :], 0.0)

            def stage_x(c):
                """DMA chunk c's x into staging buffer c%2 (two quadrants:
                rows 0:33 -> B-blocks (odd 32-col groups), rows 33:66 ->
                A-blocks (even groups))."""
                buf = xs_bufs[c % 2]
                t0 = c * S
                dstB = buf[0:KA, :].rearrange(
                    "p (s c) -> p s c", c=2 * B)[:, :, B:2 * B]
                nc.sync.dma_start(dstB, xcat[0:KA, t0:t0 + S, :])
                dstA = buf[KA:2 * KA, :].rearrange(
                    "p (s c) -> p s c", c=2 * B)[:, :, 0:B]
                nc.sync.dma_start(dstA, xcat[KA:2 * KA, t0:t0 + S, :])

            def out_dma(c):
                """Slot j of buffer c%2 holds h_{cS+j-1}; stream slots
                1..S-1 (= h_{cS}..h_{cS+S-2}) to DRAM."""
                buf = rhs_bufs[c % 2]
                src = buf[64:128, :].rearrange(
                    "p (s c) -> p s c", c=2 * B)[:, 1:S, 0:B]
                nc.sync.dma_start(hout[:, c * S:c * S + S - 1, :], src)

            def out_dma_tail(c):
                """h_{cS+S-1} lands in buffer (c+1)%2 slot 0."""
                buf = rhs_bufs[(c + 1) % 2]
                src = buf[64:128, :].rearrange(
                    "p (s c) -> p s c", c=2 * B)[:, 0, 0:B]
                nc.sync.dma_start(hout[:, c * S + S - 1, :], src)

            stage_x(0)
            if n_chunk > 1:
                stage_x(1)

            X_cur = xpool.tile([HID, 2 * B], f32, name="X0")
            nc.vector.memset(X_cur[:, :], 0.0)   # c_{-1} = 0

            for c in range(n_chunk):
                rbuf = rhs_bufs[c % 2]
                xbuf = xs_bufs[c % 2]
                for k in range(n_bank):
                    bank = bankp.tile([128, 512], f32)
                    nc.tensor.matmul(
                        bank[:, :], lhsT=wx_ap,
                        rhs=xbuf[:, k * 512:(k + 1) * 512],
                        start=True, stop=False, skip_group_check=True)
                    for j in range(WIN2):
                        tau = k * WIN2 + j          # chunk-local step
                        t = c * S + tau             # global step
                        glob_next = t + 1
                        nrbuf = rhs_bufs[(glob_next // S) % 2]
                        nslot = glob_next % S
                        rhs_sl = rbuf[:, tau * 2 * B:(tau + 1) * 2 * B]
                        nc.tensor.matmul(
                            bank[:, j * 2 * B:(j + 1) * 2 * B],
                            lhsT=wh_ap, rhs=rhs_sl,
                            start=False, stop=(j == WIN2 - 1),
                            skip_group_check=True)
                        # T contiguous: [0:B)=chunk A (sig_i p<64, sig_o
                        # p>=64), [B:2B)=chunk B (sig_f p<64, junk p>=64)
                        Tt = tpool.tile([128, 2 * B], f32)
                        nc.scalar.activation(
                            Tt[:, :], bank[:, j * 2 * B:(j + 1) * 2 * B],
                            SIG)
                        # t_g -> X even slots (X: t_g@even, c@odd)
                        nc.scalar.activation(
                            X_cur[:, :].rearrange(
                                "p (b c) -> p c b", c=2)[:, 0, :],
                            bank[64:128, j * 2 * B + B:(j + 1) * 2 * B],
                            TANH)
                        # M = (sig_i*t_g @even, sig_f*c @odd)
                        Mt = mpool.tile([HID, 2 * B], f32)
                        nc.vector.tensor_tensor(
                            Mt[:, :],
                            Tt[0:64, :].rearrange("p (c b) -> p b c", c=2),
                            X_cur[:, :], MULT)
                        # c' = sig_i*t_g + sig_f*c -> X_next odd slots
                        X_next = xpool.tile([HID, 2 * B], f32)
                        nc.vector.tensor_tensor_scan(
                            X_next[:, :], scanc[:, :], Mt[:, :],
                            0.0, MULT, ADD)
                        wt = wtpool.tile([128, B], f32)
                        nc.scalar.activation(
                            wt[64:128, :],
                            X_next[:, :].rearrange(
                                "p (b c) -> p c b", c=2)[:, 1, :],
                            TANH)
                        # h = sig_o * w, written to BOTH quadrants of the
                        # next h-slot by DVE and Pool in parallel (both
                        # depend only on the tanh above — one serial link
                        # fewer than mult-then-copy).
                        nsl = nrbuf[:, nslot * 2 * B:(nslot + 1) * 2 * B]
                        nc.vector.tensor_tensor(
                            nsl[64:128, 0:B], Tt[64:128, 0:B],
                            wt[64:128, :], MULT)
                        nc.gpsimd.tensor_tensor(
                            nsl[0:64, B:2 * B], Tt[64:128, 0:B],
                            wt[64:128, :], MULT)
                        X_cur = X_next
                if c + 2 < n_chunk:
                    stage_x(c + 2)
                out_dma(c)
                out_dma_tail(c)
    return nc


def _split_waits(nc, mybir, nmax=1):
    """This walrus accepts only one sync-wait per instruction: move excess
    waits onto preceding same-engine NOPs."""
    fn = nc.m.functions[0]
    for bb in fn.blocks:
        newlist = []
        for ins in bb.instructions:
            si = getattr(ins, "sync_info", None)
            if si is not None and si.on_wait and len(si.on_wait) > nmax:
                waits = list(si.on_wait)
                while len(waits) > nmax:
                    chunk, waits = waits[:nmax], waits[nmax:]
                    nop = mybir.InstNoOp(
                        name=nc.get_next_instruction_name(), ins=[], outs=[])
                    nop.engine = ins.engine
                    nop.sync_info = mybir.SyncInfo(on_wait=chunk, on_update=[])
                    newlist.append(nop)
                si.on_wait = waits
            newlist.append(ins)
        bb.instructions[:] = newlist


def _prep_weights2(Wx, Wh, b):
    """wall [128, 256] bf16: cols 0:128 = block-diag Wh (rows 0:64 ->
    chunk B = [f;g], rows 64:128 -> chunk A = [i;o]); cols 128:256 rows
    0:66 = [Wx;b] for B then A."""
    H = HID
    idx_i = np.arange(0, H)
    idx_f = np.arange(H, 2 * H)
    idx_g = np.arange(2 * H, 3 * H)
    idx_o = np.arange(3 * H, 4 * H)
    A = np.concatenate([idx_i, idx_o])
    Bo = np.concatenate([idx_f, idx_g])
    Whf = np.asarray(Wh, np.float32)
    Wxa = np.concatenate([np.asarray(Wx, np.float32),
                          np.asarray(b, np.float32)[None, :]], axis=0)
    wall = np.zeros((128, 256), np.float32)
    wall[0:64, 0:128] = Whf[:, Bo]
    wall[64:128, 0:128] = Whf[:, A]
    wall[0:KA, 128:256] = Wxa[:, Bo]
    wall[KA:2 * KA, 128:256] = Wxa[:, A]
    return wall.astype(BF16)


def _prep_x2(y_core):
    """y_core [BPC, T, OBS] -> xcat [66, T, BPC] bf16 ([x;1] twice)."""
    t_steps = y_core.shape[1]
    xt = y_core.transpose(2, 1, 0)  # [OBS, T, BPC]
    xa = np.empty((2 * KA, t_steps, BPC), np.float32)
    xa[0:OBS] = xt
    xa[OBS] = 1.0
    xa[KA:KA + OBS] = xt
    xa[KA + OBS] = 1.0
    return np.ascontiguousarray(xa.astype(BF16))


def kernel(y, Wx, Wh, b):
    from concourse.bass_utils import run_bass_kernel_spmd

    y = np.asarray(y)
    t_steps = y.shape[1]
    wall = _prep_weights2(Wx, Wh, b)
    key = ("v2", t_steps)
    if key not in _NC_CACHE:
        import concourse.mybir as mybir
        nc = build_nc2(t_steps)
        _split_waits(nc, mybir)
        _NC_CACHE[key] = nc
    nc = _NC_CACHE[key]
    in_maps = [{"wall": wall, "xcat": _prep_x2(y[c * BPC:(c + 1) * BPC])}
               for c in range(N_CORES)]
    globals()["_LAST_IN_MAPS"] = in_maps
    res = run_bass_kernel_spmd(
        nc, in_maps, core_ids=list(range(N_CORES)),
        trace=bool(int(os.environ.get("LSTM_TRACE", "0"))))
    out = np.empty((B_FULL, t_steps, HID), np.float32)
    for c in range(N_CORES):
        hg = res.results[c]["hout"].astype(np.float32)  # [HID, T, BPC]
        out[c * BPC:(c + 1) * BPC] = hg.transpose(2, 1, 0)
    globals()["_LAST_RESULT"] = res
    return out
